# revision 27
# baseline (speedup 1.0000x reference)
"""Trainium2 Bass kernel for nn_DeepSSM_Net (PointNet++-style SSM head).

Pure data parallel: B=128 samples, 16 per core across 8 NeuronCores.
Per core: xyz planes SBUF-resident as [128 partitions = 16 samples x 8
groups, 8192 points]. Fused FPS (10 centers) + ball query:

Per step, per 1024-col chunk:
  - ScalarE: 3 Square activations (x-cx)^2 etc (bias = -c per partition)
  - GpSimd:  sqx+sqy (scalar_tensor_tensor), half the code passes
  - VectorE: d = (sqx+sqy)+sqz, fused min-update + chunk-max
    (tensor_tensor_reduce), per-chunk argmax index (max_index), top-8
    in-radius codes (InstMax on f16 codes).
Argmax resolution across the 8 partition-groups of a sample is done
on-chip with DVE 32x32 stream transposes (no DRAM bounce); the winner's
coords are gathered per-partition with one indirect DMA per coordinate,
which also broadcasts the new center to all 8 group partitions.

Ball-query codes: per chunk top-8 of (d <= R2) * (1024-col) in f16,
merged at the end into per-sample first-8-by-index codes (f32 integer
code space), decoded to indices on host. Tiny MLP/BN/FC head runs on
host (~0.003% of FLOPs, couples samples across cores through batch BN).
"""

import numpy as np

# problem constants (hardcoded per the task contract)
B, N, S, NSAMP = 128, 65536, 10, 8
NCORES, SPC = 8, 16          # cores, samples per core
G, FPP = 8, 8192             # partition-groups per sample, points/partition
CH, NCH = 1024, 8            # chunk columns, chunks per row
R2 = float(np.float32(0.04))
CSTEP = 1025                 # chunk-code stride (code in [1,1024], 0 empty)
GSTEP = 8201                 # group-code stride (chunk code in [1,8200])
MAXF = float(B * FPP)        # 1048576: > any flat index
WBIG = -3.0e38

_CACHE = {}


def _build_program():
    import os
    import concourse.bass as bass
    import concourse.tile as tile
    from concourse import bacc, mybir
    from concourse._compat import with_exitstack

    stage = int(os.environ.get("KERNEL_STAGE", "4"))
    chunk_ops = int(os.environ.get("KERNEL_CHUNK_OPS", "3"))

    F32 = mybir.dt.float32
    F16 = mybir.dt.float16
    U16 = mybir.dt.uint16
    U32 = mybir.dt.uint32
    Alu = mybir.AluOpType
    Act = mybir.ActivationFunctionType
    Ax = mybir.AxisListType

    nc = bacc.Bacc("TRN2", target_bir_lowering=False, debug=False,
                   num_devices=1)

    xd = nc.dram_tensor("xd", [128, FPP], F32, kind="ExternalInput")
    yd = nc.dram_tensor("yd", [128, FPP], F32, kind="ExternalInput")
    zd = nc.dram_tensor("zd", [128, FPP], F32, kind="ExternalInput")
    idxvc_d = nc.dram_tensor("idxvc", [128, CH], F16, kind="ExternalInput")
    cs0_d = nc.dram_tensor("cs0", [128, 3], F32, kind="ExternalInput")
    iota8_d = nc.dram_tensor("iota8", [128, G], F32, kind="ExternalInput")
    pbase_d = nc.dram_tensor("pbase", [128, 1], F32, kind="ExternalInput")
    addc_d = nc.dram_tensor("addc", [128, NCH * NSAMP], F32,
                            kind="ExternalInput")
    gofs_d = nc.dram_tensor("gofs", [128, 1], F32, kind="ExternalInput")
    newxyz_d = nc.dram_tensor("newxyz", [128, 3 * S], F32,
                              kind="ExternalOutput")
    vout_d = nc.dram_tensor("vout", [SPC, NSAMP * S], F32,
                            kind="ExternalOutput")

    @with_exitstack
    def prog(ctx, tc):
        big = ctx.enter_context(tc.tile_pool(name="big", bufs=1))
        scr = ctx.enter_context(tc.tile_pool(name="scr", bufs=2))
        cod = ctx.enter_context(tc.tile_pool(name="cod", bufs=2))
        sml = ctx.enter_context(tc.tile_pool(name="sml", bufs=2))
        cst = ctx.enter_context(tc.tile_pool(name="cst", bufs=1))
        csp = ctx.enter_context(tc.tile_pool(name="csp", bufs=2))
        drm = ctx.enter_context(tc.tile_pool(name="drm", bufs=1,
                                             space="DRAM"))

        # pad between big tiles: breaks 32KB address aliasing between the
        # 2-read+1-write streams of the min op (measured 3x slowdown when
        # DD/DST are exactly 32KB apart)
        X = big.tile([128, FPP], F32, tag="X",
                     padded_shape=[128, FPP + 136])
        Y = big.tile([128, FPP], F32, tag="Y",
                     padded_shape=[128, FPP + 104])
        Z = big.tile([128, FPP], F32, tag="Z",
                     padded_shape=[128, FPP + 136])
        DST = big.tile([128, FPP], F32, tag="DST",
                       padded_shape=[128, FPP + 104])
        DD = big.tile([128, FPP], F32, tag="DD",
                      padded_shape=[128, FPP + 136])
        # interleave half-plane loads so step-0 compute starts early
        H = FPP // 2
        for h in range(2):
            hs = slice(h * H, (h + 1) * H)
            nc.sync.dma_start(X[:, hs], xd.ap()[:, hs])
            nc.sync.dma_start(Y[:, hs], yd.ap()[:, hs])
            nc.sync.dma_start(Z[:, hs], zd.ap()[:, hs])
        nc.vector.memset(DST[:], 1.0e10)

        idxvc = cst.tile([128, CH], F16, tag="idxvc")
        nc.sync.dma_start(idxvc[:], idxvc_d.ap())
        iota8 = cst.tile([128, G], F32, tag="iota8")
        nc.sync.dma_start(iota8[:], iota8_d.ap())
        pbase = cst.tile([128, 1], F32, tag="pbase")
        nc.sync.dma_start(pbase[:], pbase_d.ap())
        addc = cst.tile([128, NCH * NSAMP], F32, tag="addc")
        nc.sync.dma_start(addc[:], addc_d.ap())
        gofs = cst.tile([128, 1], F32, tag="gofs")
        nc.sync.dma_start(gofs[:], gofs_d.ap())

        ones8 = cst.tile([128, G], F32, tag="ones8")
        nc.vector.memset(ones8[:], 1.0)
        r2c = cst.tile([128, 1], F32, tag="r2c")
        nc.vector.memset(r2c[:], R2)
        vt8 = cst.tile([128, S * NCH * NSAMP], F16, tag="vt8")
        p8all = cst.tile([128, S * NSAMP], F32, tag="p8all")
        nxyz = cst.tile([128, 3 * S], F32, tag="nxyz")
        nc.vector.memset(nxyz[:, 0:3], 0.0)
        pack = cst.tile([128, 64], F32, tag="pack")
        nc.vector.memset(pack[:], 0.0)
        packB = cst.tile([128, 32], F32, tag="packB")

        cs0 = csp.tile([128, 3], F32, tag="csneg")
        nc.sync.dma_start(cs0[:], cs0_d.ap())
        csneg = cs0

        xyz_flat = [
            bass.AP(t.ap().tensor, 0, [[1, 128 * FPP], [1, 1]])
            for t in (xd, yd, zd)
        ]

        def emit_bq(k):
            if stage < 3:
                return
            # deferred ball-query codes + top8 for step k (reads DD)
            for j in range(NCH):
                sl = slice(j * CH, (j + 1) * CH)
                V16 = cod.tile([128, CH], F16, tag="V16",
                               padded_shape=[128, CH + 16])
                # fused (d <= R2) * codes: one pass, no SG round-trip
                # (the kernel is SBUF-bandwidth-bound, so fewer bytes wins)
                nc.vector.scalar_tensor_tensor(V16[:], DD[:, sl], R2,
                                               idxvc[:], Alu.is_le, Alu.mult)
                nc.vector.max(vt8[:, (k * NCH + j) * 8:(k * NCH + j) * 8 + 8],
                              V16[:])

        for k in range(S):
            cm8 = sml.tile([128, NCH * 8], F32, tag="cm8")
            cidx = sml.tile([128, NCH * 8], U16, tag="cidx")
            for j in range(NCH):
                sl = slice(j * CH, (j + 1) * CH)
                A = scr.tile([128, CH], F32, tag="A",
                             padded_shape=[128, CH + 8])
                Bt = scr.tile([128, CH], F32, tag="B",
                              padded_shape=[128, CH + 16])
                Ct = scr.tile([128, CH], F32, tag="C",
                              padded_shape=[128, CH + 8])
                nc.scalar.activation(A[:], X[:, sl], Act.Square,
                                     bias=csneg[:, 0:1])
                nc.scalar.activation(Bt[:], Y[:, sl], Act.Square,
                                     bias=csneg[:, 1:2])
                nc.scalar.activation(Ct[:], Z[:, sl], Act.Square,
                                     bias=csneg[:, 2:3])
                nc.gpsimd.tensor_tensor(A[:], A[:], Bt[:], Alu.add)
                nc.gpsimd.tensor_tensor(DD[:, sl], A[:], Ct[:], Alu.add)
                if chunk_ops < 2:
                    continue
                # dist = min(d, dist)
                nc.vector.tensor_tensor(DST[:, sl], DD[:, sl], DST[:, sl],
                                        Alu.min)
                if chunk_ops < 3:
                    continue
                # chunk max + its in-chunk index
                nc.vector.max(cm8[:, j * 8:(j + 1) * 8], DST[:, sl])
                nc.vector.max_index(cidx[:, j * 8:(j + 1) * 8],
                                    cm8[:, j * 8:(j + 1) * 8], DST[:, sl])
            if k == S - 1:
                emit_bq(k)
                break
            if stage < 2:
                continue
            # ---- argmax tail: resolve winner across chunks+groups ----
            cmax = cm8.rearrange("p (j e) -> p j e", e=8)[:, :, 0]
            rmax = sml.tile([128, 1], F32, tag="rmax")
            nc.vector.tensor_reduce(rmax[:], cmax, Ax.X, Alu.max)
            rm8b = sml.tile([128, 8], F32, tag="rm8b")
            nc.gpsimd.tensor_scalar(rm8b[:], ones8[:], rmax[:, 0:1], None,
                                    op0=Alu.mult)
            cfind = sml.tile([128, 8], U16, tag="cfind")
            nc.vector.max_index(cfind[:], rm8b[:], cmax)
            cf = sml.tile([128, 1], F32, tag="cf")
            nc.vector.tensor_copy(cf[:], cfind[:, 0:1])
            oh = sml.tile([128, 8], F32, tag="oh")
            nc.vector.tensor_scalar(oh[:], iota8[:], cf[:, 0:1], None,
                                    op0=Alu.is_equal)
            cidxf = sml.tile([128, 8], F32, tag="cidxf")
            nc.vector.tensor_copy(
                cidxf[:], cidx.rearrange("p (j e) -> p j e", e=8)[:, :, 0])
            ohp = sml.tile([128, 8], F32, tag="ohp")
            idxin = sml.tile([128, 1], F32, tag="idxin")
            nc.vector.tensor_tensor(ohp[:], oh[:], cidxf[:], Alu.mult)
            nc.vector.tensor_reduce(idxin[:], ohp[:], Ax.X, Alu.add)
            flat = sml.tile([128, 1], F32, tag="flat")
            nc.vector.scalar_tensor_tensor(flat[:], cf[:], float(CH),
                                           idxin[:], Alu.mult, Alu.add)
            nc.vector.tensor_tensor(flat[:], flat[:], pbase[:], Alu.add)
            nc.vector.tensor_copy(pack[:, 0:1], rmax[:])
            nc.vector.tensor_copy(pack[:, 32:33], flat[:])
            T = sml.tile([128, 64], F32, tag="T")
            nc.vector.transpose(T[:], pack[:])
            Tv = T.rearrange("p (q r) -> p q r", r=8)
            smax4 = sml.tile([128, 4], F32, tag="smax4")
            nc.vector.tensor_reduce(smax4[:], Tv[:, 0:4, :], Ax.X, Alu.max)
            smaxb = sml.tile([128, 32], F32, tag="smaxb")
            sbv = smaxb.rearrange("p (q r) -> p q r", r=8)
            for r in range(8):
                nc.vector.tensor_copy(sbv[:, :, r], smax4[:])
            mask = sml.tile([128, 32], F32, tag="mask")
            nc.vector.tensor_tensor(mask[:], T[:, 0:32], smaxb[:], Alu.is_ge)
            sub = sml.tile([128, 32], F32, tag="sub")
            nc.vector.tensor_scalar(sub[:], T[:, 32:64], -1.0, MAXF,
                                    op0=Alu.mult, op1=Alu.add)
            enc = sml.tile([128, 32], F32, tag="enc")
            nc.vector.tensor_mul(enc[:], mask[:], sub[:])
            e4 = sml.tile([128, 4], F32, tag="e4")
            nc.vector.tensor_reduce(
                e4[:], enc.rearrange("p (q r) -> p q r", r=8)[:], Ax.X,
                Alu.max)
            wf4 = sml.tile([128, 4], F32, tag="wf4")
            nc.vector.tensor_scalar(wf4[:], e4[:], -1.0, MAXF,
                                    op0=Alu.mult, op1=Alu.add)
            pbv = packB.rearrange("p (q r) -> p q r", r=8)
            for r in range(8):
                nc.vector.tensor_copy(pbv[:, :, r], wf4[:])
            TB = sml.tile([128, 32], F32, tag="TB")
            nc.vector.transpose(TB[:], packB[:])
            flatu = sml.tile([128, 1], U32, tag="flatu")
            nc.vector.tensor_copy(flatu[:], TB[:, 0:1])
            cs = csp.tile([128, 3], F32, tag="cs")
            for c, fl in enumerate(xyz_flat):
                nc.gpsimd.indirect_dma_start(
                    cs[:, c:c + 1], None, fl,
                    bass.IndirectOffsetOnAxis(ap=flatu[:], axis=0))
            col = 3 * (k + 1)
            nc.vector.tensor_copy(nxyz[:, col:col + 3], cs[:])
            csneg = csp.tile([128, 3], F32, tag="csneg")
            nc.vector.tensor_scalar(csneg[:], cs[:], -1.0, None, op0=Alu.mult)
            emit_bq(k)

        # ---- ball-query merge ----
        if stage < 4:
            vout0 = cst.tile([SPC, NSAMP * S], F32, tag="vout")
            nc.vector.memset(vout0[:], 0.0)
            nc.sync.dma_start(newxyz_d.ap(), nxyz[:])
            nc.sync.dma_start(vout_d.ap(), vout0[:])
            return
        for k in range(S):
            w64 = sml.tile([128, 64], F32, tag="w64")
            nc.vector.tensor_copy(w64[:], vt8[:, k * 64:(k + 1) * 64])
            t64 = sml.tile([128, 64], F32, tag="t64")
            nc.vector.tensor_tensor(t64[:], w64[:], addc[:], Alu.add)
            g64 = sml.tile([128, 64], F32, tag="g64")
            nc.vector.scalar_tensor_tensor(g64[:], w64[:], 0.0, t64[:],
                                           Alu.is_gt, Alu.mult)
            p8k = sml.tile([128, 8], F32, tag="p8k")
            nc.vector.max(p8k[:], g64[:])
            tp8 = sml.tile([128, 8], F32, tag="tp8")
            nc.vector.tensor_scalar(tp8[:], p8k[:], gofs[:, 0:1], None,
                                    op0=Alu.add)
            nc.vector.scalar_tensor_tensor(p8all[:, k * 8:k * 8 + 8], p8k[:],
                                           0.0, tp8[:], Alu.is_gt, Alu.mult)
        dp8 = drm.tile([128, S * NSAMP], F32, tag="dp8")
        nc.sync.dma_start(dp8[:], p8all[:])
        sc = cst.tile([SPC, G * S * NSAMP], F32, tag="sc")
        nc.sync.dma_start(sc[:], dp8.rearrange("(s g) c -> s (g c)", g=G))
        scv = sc.rearrange("s (g c) -> s g c", c=S * NSAMP)
        vout = cst.tile([SPC, NSAMP * S], F32, tag="vout")
        for k in range(S):
            nc.vector.max(vout[:, k * 8:k * 8 + 8],
                          scv[:, :, k * 8:k * 8 + 8])
        nc.sync.dma_start(newxyz_d.ap(), nxyz[:])
        nc.sync.dma_start(vout_d.ap(), vout[:])

    with tile.TileContext(nc) as tc:
        prog(tc)
    nc.compile()
    return nc


def _get_nc():
    if "nc" not in _CACHE:
        _CACHE["nc"] = _build_program()
    return _CACHE["nc"]


def _make_consts():
    idxvc = np.broadcast_to(
        (CH - np.arange(CH, dtype=np.float16))[None, :].astype(np.float16),
        (128, CH)).copy()
    iota8 = np.broadcast_to(
        np.arange(G, dtype=np.float32)[None, :], (128, G)).copy()
    pbase = (np.arange(128, dtype=np.float32) * FPP)[:, None].copy()
    cols = np.arange(NCH * NSAMP)
    addc = np.broadcast_to(
        ((NCH - 1 - cols // NSAMP) * CSTEP).astype(np.float32)[None, :],
        (128, NCH * NSAMP)).copy()
    gofs = ((G - 1 - np.arange(128) % G) * GSTEP).astype(
        np.float32)[:, None].copy()
    return idxvc, iota8, pbase, addc, gofs


def _make_in_maps(pc):
    idxvc, iota8, pbase, addc, gofs = _make_consts()
    in_maps = []
    for i in range(NCORES):
        shard = pc[i * SPC:(i + 1) * SPC]          # [16, 3, 65536]
        planes = [np.ascontiguousarray(
            shard[:, c, :].reshape(128, FPP)) for c in range(3)]
        p0 = shard[:, :, 0]                        # [16, 3]
        cs0 = np.repeat(-p0, G, axis=0).astype(np.float32)   # [128, 3]
        in_maps.append({
            "xd": planes[0], "yd": planes[1], "zd": planes[2],
            "idxvc": idxvc, "cs0": cs0, "iota8": iota8,
            "pbase": pbase, "addc": addc, "gofs": gofs,
        })
    return in_maps


def _decode_neighbors(vout):
    """vout: [B, S, 8] merged sample-codes -> idx [B, S, 8] int32."""
    u = np.rint(vout).astype(np.int64)
    g = (G - 1) - (np.maximum(u, 1) - 1) // GSTEP
    rem = u - (G - 1 - g) * GSTEP
    j = (NCH - 1) - (np.maximum(rem, 1) - 1) // CSTEP
    code = rem - (NCH - 1 - j) * CSTEP
    col = CH - code
    n = g * FPP + j * CH + col
    empty = u == 0
    n = np.where(empty, n[:, :, 0:1], n)
    return n.astype(np.int32)


def _host_head(pc, new_xyz, idx, p):
    """grouping + shared MLP + BN + FC head (numpy, float64 accum)."""
    xyz = pc.transpose(0, 2, 1).astype(np.float64)       # [B, N, 3]
    bi = np.arange(B)[:, None, None]
    grouped = xyz[bi, idx]                               # [B, S, 8, 3]
    grouped = grouped - new_xyz[:, :, None, :].astype(np.float64)
    x = grouped.transpose(0, 3, 2, 1)                    # [B, 3, 8, S]

    def bn(v, g, be):
        m = v.mean(axis=(0, 2, 3), keepdims=True)
        var = v.var(axis=(0, 2, 3), keepdims=True)
        return (v - m) / np.sqrt(var + 1e-5) * g[None, :, None, None] \
            + be[None, :, None, None]

    for w, b, g, be in (("w1", "b1", "g1", "be1"), ("w2", "b2", "g2", "be2"),
                        ("w3", "b3", "g3", "be3")):
        w, b, g, be = (p[w].astype(np.float64), p[b].astype(np.float64),
                       p[g].astype(np.float64), p[be].astype(np.float64))
        x = np.einsum("oc,bcns->bons", w, x) + b[None, :, None, None]
        x = np.maximum(bn(x, g, be), 0.0)
    x = x.max(axis=2)                                    # [B, 16, S]
    feat = x.reshape(B, -1)
    h = feat @ p["fc1_w"].astype(np.float64).T + p["fc1_b"].astype(np.float64)
    m = h.mean(0, keepdims=True)
    v = h.var(0, keepdims=True)
    h = (h - m) / np.sqrt(v + 1e-5) * p["bn1_g"].astype(np.float64) \
        + p["bn1_b"].astype(np.float64)
    h = np.maximum(h, 0.0)
    out = h @ p["fc2_w"].astype(np.float64).T + p["fc2_b"].astype(np.float64)
    return out.astype(np.float32)


def run_device(pc, trace=False, return_raw=False):
    """Returns (new_xyz [B,S,3] f32, idx [B,S,8] i32) from the 8-core run."""
    from concourse import bass_utils
    nc = _get_nc()
    in_maps = _make_in_maps(pc)
    res = bass_utils.run_bass_kernel_spmd(nc, in_maps,
                                          core_ids=list(range(NCORES)),
                                          trace=trace)
    new_xyz = np.zeros((B, S, 3), np.float32)
    vout = np.zeros((B, S, NSAMP), np.float32)
    for i in range(NCORES):
        r = res.results[i]
        new_xyz[i * SPC:(i + 1) * SPC] = \
            r["newxyz"][::G].reshape(SPC, S, 3)
        vout[i * SPC:(i + 1) * SPC] = r["vout"].reshape(SPC, S, NSAMP)
    # slot 0 center comes from host (point 0 of each sample)
    new_xyz[:, 0, :] = pc[:, :, 0]
    idx = _decode_neighbors(vout)
    if return_raw:
        return new_xyz, idx, res
    return new_xyz, idx


def kernel(**inputs):
    pc = np.ascontiguousarray(inputs["pc_electrode"], dtype=np.float32)
    new_xyz, idx = run_device(pc)
    return _host_head(pc, new_xyz, idx, inputs)


# revision 29
# speedup vs baseline: 1.0561x; 1.0561x over previous
"""Trainium2 Bass kernel for nn_DeepSSM_Net (PointNet++-style SSM head).

Pure data parallel: B=128 samples, 16 per core across 8 NeuronCores.
Per core: xyz planes SBUF-resident as [128 partitions = 16 samples x 8
groups, 8192 points]. Fused FPS (10 centers) + ball query:

Per step, per 1024-col chunk:
  - ScalarE: 3 Square activations (x-cx)^2 etc (bias = -c per partition)
  - GpSimd:  sqx+sqy (scalar_tensor_tensor), half the code passes
  - VectorE: d = (sqx+sqy)+sqz, fused min-update + chunk-max
    (tensor_tensor_reduce), per-chunk argmax index (max_index), top-8
    in-radius codes (InstMax on f16 codes).
Argmax resolution across the 8 partition-groups of a sample is done
on-chip with DVE 32x32 stream transposes (no DRAM bounce); the winner's
coords are gathered per-partition with one indirect DMA per coordinate,
which also broadcasts the new center to all 8 group partitions.

Ball-query codes: per chunk top-8 of (d <= R2) * (1024-col) in f16,
merged at the end into per-sample first-8-by-index codes (f32 integer
code space), decoded to indices on host. Tiny MLP/BN/FC head runs on
host (~0.003% of FLOPs, couples samples across cores through batch BN).
"""

import numpy as np

# problem constants (hardcoded per the task contract)
B, N, S, NSAMP = 128, 65536, 10, 8
NCORES, SPC = 8, 16          # cores, samples per core
G, FPP = 8, 8192             # partition-groups per sample, points/partition
CH, NCH = 1024, 8            # chunk columns, chunks per row
R2 = float(np.float32(0.04))
CSTEP = 1025                 # chunk-code stride (code in [1,1024], 0 empty)
GSTEP = 8201                 # group-code stride (chunk code in [1,8200])
MAXF = float(B * FPP)        # 1048576: > any flat index
WBIG = -3.0e38

_CACHE = {}


def _build_program():
    import os
    import concourse.bass as bass
    import concourse.tile as tile
    from concourse import bacc, mybir
    from concourse._compat import with_exitstack

    stage = int(os.environ.get("KERNEL_STAGE", "4"))
    chunk_ops = int(os.environ.get("KERNEL_CHUNK_OPS", "3"))

    F32 = mybir.dt.float32
    F16 = mybir.dt.float16
    U16 = mybir.dt.uint16
    U32 = mybir.dt.uint32
    Alu = mybir.AluOpType
    Act = mybir.ActivationFunctionType
    Ax = mybir.AxisListType

    nc = bacc.Bacc("TRN2", target_bir_lowering=False, debug=False,
                   num_devices=1)

    xd = nc.dram_tensor("xd", [128, FPP], F32, kind="ExternalInput")
    yd = nc.dram_tensor("yd", [128, FPP], F32, kind="ExternalInput")
    zd = nc.dram_tensor("zd", [128, FPP], F32, kind="ExternalInput")
    idxvc_d = nc.dram_tensor("idxvc", [128, CH], F16, kind="ExternalInput")
    cs0_d = nc.dram_tensor("cs0", [128, 3], F32, kind="ExternalInput")
    iota8_d = nc.dram_tensor("iota8", [128, G], F32, kind="ExternalInput")
    pbase_d = nc.dram_tensor("pbase", [128, 1], F32, kind="ExternalInput")
    addc_d = nc.dram_tensor("addc", [128, NCH * NSAMP], F32,
                            kind="ExternalInput")
    gofs_d = nc.dram_tensor("gofs", [128, 1], F32, kind="ExternalInput")
    newxyz_d = nc.dram_tensor("newxyz", [128, 3 * S], F32,
                              kind="ExternalOutput")
    vout_d = nc.dram_tensor("vout", [SPC, NSAMP * S], F32,
                            kind="ExternalOutput")

    @with_exitstack
    def prog(ctx, tc):
        big = ctx.enter_context(tc.tile_pool(name="big", bufs=1))
        scr = ctx.enter_context(tc.tile_pool(name="scr", bufs=2))
        cod = ctx.enter_context(tc.tile_pool(name="cod", bufs=2))
        sml = ctx.enter_context(tc.tile_pool(name="sml", bufs=2))
        cst = ctx.enter_context(tc.tile_pool(name="cst", bufs=1))
        csp = ctx.enter_context(tc.tile_pool(name="csp", bufs=2))
        drm = ctx.enter_context(tc.tile_pool(name="drm", bufs=1,
                                             space="DRAM"))

        # pad between big tiles: breaks 32KB address aliasing between the
        # 2-read+1-write streams of the min op (measured 3x slowdown when
        # DD/DST are exactly 32KB apart)
        X = big.tile([128, FPP], F32, tag="X",
                     padded_shape=[128, FPP + 136])
        Y = big.tile([128, FPP], F32, tag="Y",
                     padded_shape=[128, FPP + 104])
        Z = big.tile([128, FPP], F32, tag="Z",
                     padded_shape=[128, FPP + 136])
        DST = big.tile([128, FPP], F32, tag="DST",
                       padded_shape=[128, FPP + 104])
        DD = big.tile([128, FPP], F32, tag="DD",
                      padded_shape=[128, FPP + 136])
        nc.sync.dma_start(X[:], xd.ap())
        nc.sync.dma_start(Y[:], yd.ap())
        nc.sync.dma_start(Z[:], zd.ap())
        nc.vector.memset(DST[:], 1.0e10)

        idxvc = cst.tile([128, CH], F16, tag="idxvc")
        nc.sync.dma_start(idxvc[:], idxvc_d.ap())
        iota8 = cst.tile([128, G], F32, tag="iota8")
        nc.sync.dma_start(iota8[:], iota8_d.ap())
        pbase = cst.tile([128, 1], F32, tag="pbase")
        nc.sync.dma_start(pbase[:], pbase_d.ap())
        addc = cst.tile([128, NCH * NSAMP], F32, tag="addc")
        nc.sync.dma_start(addc[:], addc_d.ap())
        gofs = cst.tile([128, 1], F32, tag="gofs")
        nc.sync.dma_start(gofs[:], gofs_d.ap())

        ones8 = cst.tile([128, G], F32, tag="ones8")
        nc.vector.memset(ones8[:], 1.0)
        r2c = cst.tile([128, 1], F32, tag="r2c")
        nc.vector.memset(r2c[:], R2)
        vt8 = cst.tile([128, S * NCH * NSAMP], F16, tag="vt8")
        p8all = cst.tile([128, S * NSAMP], F32, tag="p8all")
        nxyz = cst.tile([128, 3 * S], F32, tag="nxyz")
        nc.vector.memset(nxyz[:, 0:3], 0.0)
        pack = cst.tile([128, 64], F32, tag="pack")
        nc.vector.memset(pack[:], 0.0)
        packB = cst.tile([128, 32], F32, tag="packB")

        cs0 = csp.tile([128, 3], F32, tag="csneg")
        nc.sync.dma_start(cs0[:], cs0_d.ap())
        csneg = cs0

        xyz_flat = [
            bass.AP(t.ap().tensor, 0, [[1, 128 * FPP], [1, 1]])
            for t in (xd, yd, zd)
        ]

        def emit_bq(k):
            if stage < 3:
                return
            # deferred ball-query codes + top8 for step k (reads DD)
            for j in range(NCH):
                sl = slice(j * CH, (j + 1) * CH)
                V16 = cod.tile([128, CH], F16, tag="V16",
                               padded_shape=[128, CH + 16])
                # sign(R2 - d) on ScalarE, f16 mul on V: measured faster
                # than the fused STT form (801us vs 842us wall)
                SG = cod.tile([128, CH], F16, tag="SG",
                              padded_shape=[128, CH + 48])
                nc.scalar.activation(SG[:], DD[:, sl], Act.Sign,
                                     bias=r2c[:, 0:1], scale=-1.0)
                nc.vector.tensor_tensor(V16[:], SG[:], idxvc[:], Alu.mult)
                nc.vector.max(vt8[:, (k * NCH + j) * 8:(k * NCH + j) * 8 + 8],
                              V16[:])

        for k in range(S):
            cm8 = sml.tile([128, NCH * 8], F32, tag="cm8")
            cidx = sml.tile([128, NCH * 8], U16, tag="cidx")
            for j in range(NCH):
                sl = slice(j * CH, (j + 1) * CH)
                A = scr.tile([128, CH], F32, tag="A",
                             padded_shape=[128, CH + 8])
                Bt = scr.tile([128, CH], F32, tag="B",
                              padded_shape=[128, CH + 16])
                Ct = scr.tile([128, CH], F32, tag="C",
                              padded_shape=[128, CH + 8])
                nc.scalar.activation(A[:], X[:, sl], Act.Square,
                                     bias=csneg[:, 0:1])
                nc.scalar.activation(Bt[:], Y[:, sl], Act.Square,
                                     bias=csneg[:, 1:2])
                nc.scalar.activation(Ct[:], Z[:, sl], Act.Square,
                                     bias=csneg[:, 2:3])
                nc.gpsimd.tensor_tensor(A[:], A[:], Bt[:], Alu.add)
                nc.gpsimd.tensor_tensor(DD[:, sl], A[:], Ct[:], Alu.add)
                if chunk_ops < 2:
                    continue
                # dist = min(d, dist)
                nc.vector.tensor_tensor(DST[:, sl], DD[:, sl], DST[:, sl],
                                        Alu.min)
                if chunk_ops < 3:
                    continue
                # chunk max + its in-chunk index
                nc.vector.max(cm8[:, j * 8:(j + 1) * 8], DST[:, sl])
                nc.vector.max_index(cidx[:, j * 8:(j + 1) * 8],
                                    cm8[:, j * 8:(j + 1) * 8], DST[:, sl])
            if k == S - 1:
                emit_bq(k)
                break
            if stage < 2:
                continue
            # ---- argmax tail: resolve winner across chunks+groups ----
            cmax = cm8.rearrange("p (j e) -> p j e", e=8)[:, :, 0]
            rmax = sml.tile([128, 1], F32, tag="rmax")
            nc.vector.tensor_reduce(rmax[:], cmax, Ax.X, Alu.max)
            rm8b = sml.tile([128, 8], F32, tag="rm8b")
            nc.gpsimd.tensor_scalar(rm8b[:], ones8[:], rmax[:, 0:1], None,
                                    op0=Alu.mult)
            cfind = sml.tile([128, 8], U16, tag="cfind")
            nc.vector.max_index(cfind[:], rm8b[:], cmax)
            cf = sml.tile([128, 1], F32, tag="cf")
            nc.vector.tensor_copy(cf[:], cfind[:, 0:1])
            oh = sml.tile([128, 8], F32, tag="oh")
            nc.vector.tensor_scalar(oh[:], iota8[:], cf[:, 0:1], None,
                                    op0=Alu.is_equal)
            cidxf = sml.tile([128, 8], F32, tag="cidxf")
            nc.vector.tensor_copy(
                cidxf[:], cidx.rearrange("p (j e) -> p j e", e=8)[:, :, 0])
            ohp = sml.tile([128, 8], F32, tag="ohp")
            idxin = sml.tile([128, 1], F32, tag="idxin")
            nc.vector.tensor_tensor(ohp[:], oh[:], cidxf[:], Alu.mult)
            nc.vector.tensor_reduce(idxin[:], ohp[:], Ax.X, Alu.add)
            flat = sml.tile([128, 1], F32, tag="flat")
            nc.vector.scalar_tensor_tensor(flat[:], cf[:], float(CH),
                                           idxin[:], Alu.mult, Alu.add)
            nc.vector.tensor_tensor(flat[:], flat[:], pbase[:], Alu.add)
            nc.vector.tensor_copy(pack[:, 0:1], rmax[:])
            nc.vector.tensor_copy(pack[:, 32:33], flat[:])
            T = sml.tile([128, 64], F32, tag="T")
            nc.vector.transpose(T[:], pack[:])
            Tv = T.rearrange("p (q r) -> p q r", r=8)
            smax4 = sml.tile([128, 4], F32, tag="smax4")
            nc.vector.tensor_reduce(smax4[:], Tv[:, 0:4, :], Ax.X, Alu.max)
            smaxb = sml.tile([128, 32], F32, tag="smaxb")
            sbv = smaxb.rearrange("p (q r) -> p q r", r=8)
            for r in range(8):
                nc.vector.tensor_copy(sbv[:, :, r], smax4[:])
            mask = sml.tile([128, 32], F32, tag="mask")
            nc.vector.tensor_tensor(mask[:], T[:, 0:32], smaxb[:], Alu.is_ge)
            sub = sml.tile([128, 32], F32, tag="sub")
            nc.vector.tensor_scalar(sub[:], T[:, 32:64], -1.0, MAXF,
                                    op0=Alu.mult, op1=Alu.add)
            enc = sml.tile([128, 32], F32, tag="enc")
            nc.vector.tensor_mul(enc[:], mask[:], sub[:])
            e4 = sml.tile([128, 4], F32, tag="e4")
            nc.vector.tensor_reduce(
                e4[:], enc.rearrange("p (q r) -> p q r", r=8)[:], Ax.X,
                Alu.max)
            wf4 = sml.tile([128, 4], F32, tag="wf4")
            nc.vector.tensor_scalar(wf4[:], e4[:], -1.0, MAXF,
                                    op0=Alu.mult, op1=Alu.add)
            pbv = packB.rearrange("p (q r) -> p q r", r=8)
            for r in range(8):
                nc.vector.tensor_copy(pbv[:, :, r], wf4[:])
            TB = sml.tile([128, 32], F32, tag="TB")
            nc.vector.transpose(TB[:], packB[:])
            flatu = sml.tile([128, 1], U32, tag="flatu")
            nc.vector.tensor_copy(flatu[:], TB[:, 0:1])
            cs = csp.tile([128, 3], F32, tag="cs")
            for c, fl in enumerate(xyz_flat):
                nc.gpsimd.indirect_dma_start(
                    cs[:, c:c + 1], None, fl,
                    bass.IndirectOffsetOnAxis(ap=flatu[:], axis=0))
            col = 3 * (k + 1)
            nc.vector.tensor_copy(nxyz[:, col:col + 3], cs[:])
            csneg = csp.tile([128, 3], F32, tag="csneg")
            nc.vector.tensor_scalar(csneg[:], cs[:], -1.0, None, op0=Alu.mult)
            emit_bq(k)

        # ---- ball-query merge ----
        if stage < 4:
            vout0 = cst.tile([SPC, NSAMP * S], F32, tag="vout")
            nc.vector.memset(vout0[:], 0.0)
            nc.sync.dma_start(newxyz_d.ap(), nxyz[:])
            nc.sync.dma_start(vout_d.ap(), vout0[:])
            return
        for k in range(S):
            w64 = sml.tile([128, 64], F32, tag="w64")
            nc.vector.tensor_copy(w64[:], vt8[:, k * 64:(k + 1) * 64])
            t64 = sml.tile([128, 64], F32, tag="t64")
            nc.vector.tensor_tensor(t64[:], w64[:], addc[:], Alu.add)
            g64 = sml.tile([128, 64], F32, tag="g64")
            nc.vector.scalar_tensor_tensor(g64[:], w64[:], 0.0, t64[:],
                                           Alu.is_gt, Alu.mult)
            p8k = sml.tile([128, 8], F32, tag="p8k")
            nc.vector.max(p8k[:], g64[:])
            tp8 = sml.tile([128, 8], F32, tag="tp8")
            nc.vector.tensor_scalar(tp8[:], p8k[:], gofs[:, 0:1], None,
                                    op0=Alu.add)
            nc.vector.scalar_tensor_tensor(p8all[:, k * 8:k * 8 + 8], p8k[:],
                                           0.0, tp8[:], Alu.is_gt, Alu.mult)
        dp8 = drm.tile([128, S * NSAMP], F32, tag="dp8")
        nc.sync.dma_start(dp8[:], p8all[:])
        sc = cst.tile([SPC, G * S * NSAMP], F32, tag="sc")
        nc.sync.dma_start(sc[:], dp8.rearrange("(s g) c -> s (g c)", g=G))
        scv = sc.rearrange("s (g c) -> s g c", c=S * NSAMP)
        vout = cst.tile([SPC, NSAMP * S], F32, tag="vout")
        for k in range(S):
            nc.vector.max(vout[:, k * 8:k * 8 + 8],
                          scv[:, :, k * 8:k * 8 + 8])
        nc.sync.dma_start(newxyz_d.ap(), nxyz[:])
        nc.sync.dma_start(vout_d.ap(), vout[:])

    with tile.TileContext(nc) as tc:
        prog(tc)
    nc.compile()
    return nc


def _get_nc():
    if "nc" not in _CACHE:
        _CACHE["nc"] = _build_program()
    return _CACHE["nc"]


def _make_consts():
    idxvc = np.broadcast_to(
        (CH - np.arange(CH, dtype=np.float16))[None, :].astype(np.float16),
        (128, CH)).copy()
    iota8 = np.broadcast_to(
        np.arange(G, dtype=np.float32)[None, :], (128, G)).copy()
    pbase = (np.arange(128, dtype=np.float32) * FPP)[:, None].copy()
    cols = np.arange(NCH * NSAMP)
    addc = np.broadcast_to(
        ((NCH - 1 - cols // NSAMP) * CSTEP).astype(np.float32)[None, :],
        (128, NCH * NSAMP)).copy()
    gofs = ((G - 1 - np.arange(128) % G) * GSTEP).astype(
        np.float32)[:, None].copy()
    return idxvc, iota8, pbase, addc, gofs


def _make_in_maps(pc):
    idxvc, iota8, pbase, addc, gofs = _make_consts()
    in_maps = []
    for i in range(NCORES):
        shard = pc[i * SPC:(i + 1) * SPC]          # [16, 3, 65536]
        planes = [np.ascontiguousarray(
            shard[:, c, :].reshape(128, FPP)) for c in range(3)]
        p0 = shard[:, :, 0]                        # [16, 3]
        cs0 = np.repeat(-p0, G, axis=0).astype(np.float32)   # [128, 3]
        in_maps.append({
            "xd": planes[0], "yd": planes[1], "zd": planes[2],
            "idxvc": idxvc, "cs0": cs0, "iota8": iota8,
            "pbase": pbase, "addc": addc, "gofs": gofs,
        })
    return in_maps


def _decode_neighbors(vout):
    """vout: [B, S, 8] merged sample-codes -> idx [B, S, 8] int32."""
    u = np.rint(vout).astype(np.int64)
    g = (G - 1) - (np.maximum(u, 1) - 1) // GSTEP
    rem = u - (G - 1 - g) * GSTEP
    j = (NCH - 1) - (np.maximum(rem, 1) - 1) // CSTEP
    code = rem - (NCH - 1 - j) * CSTEP
    col = CH - code
    n = g * FPP + j * CH + col
    empty = u == 0
    n = np.where(empty, n[:, :, 0:1], n)
    return n.astype(np.int32)


def _host_head(pc, new_xyz, idx, p):
    """grouping + shared MLP + BN + FC head (numpy, float64 accum)."""
    xyz = pc.transpose(0, 2, 1).astype(np.float64)       # [B, N, 3]
    bi = np.arange(B)[:, None, None]
    grouped = xyz[bi, idx]                               # [B, S, 8, 3]
    grouped = grouped - new_xyz[:, :, None, :].astype(np.float64)
    x = grouped.transpose(0, 3, 2, 1)                    # [B, 3, 8, S]

    def bn(v, g, be):
        m = v.mean(axis=(0, 2, 3), keepdims=True)
        var = v.var(axis=(0, 2, 3), keepdims=True)
        return (v - m) / np.sqrt(var + 1e-5) * g[None, :, None, None] \
            + be[None, :, None, None]

    for w, b, g, be in (("w1", "b1", "g1", "be1"), ("w2", "b2", "g2", "be2"),
                        ("w3", "b3", "g3", "be3")):
        w, b, g, be = (p[w].astype(np.float64), p[b].astype(np.float64),
                       p[g].astype(np.float64), p[be].astype(np.float64))
        x = np.einsum("oc,bcns->bons", w, x) + b[None, :, None, None]
        x = np.maximum(bn(x, g, be), 0.0)
    x = x.max(axis=2)                                    # [B, 16, S]
    feat = x.reshape(B, -1)
    h = feat @ p["fc1_w"].astype(np.float64).T + p["fc1_b"].astype(np.float64)
    m = h.mean(0, keepdims=True)
    v = h.var(0, keepdims=True)
    h = (h - m) / np.sqrt(v + 1e-5) * p["bn1_g"].astype(np.float64) \
        + p["bn1_b"].astype(np.float64)
    h = np.maximum(h, 0.0)
    out = h @ p["fc2_w"].astype(np.float64).T + p["fc2_b"].astype(np.float64)
    return out.astype(np.float32)


def run_device(pc, trace=False, return_raw=False):
    """Returns (new_xyz [B,S,3] f32, idx [B,S,8] i32) from the 8-core run."""
    from concourse import bass_utils
    nc = _get_nc()
    in_maps = _make_in_maps(pc)
    res = bass_utils.run_bass_kernel_spmd(nc, in_maps,
                                          core_ids=list(range(NCORES)),
                                          trace=trace)
    new_xyz = np.zeros((B, S, 3), np.float32)
    vout = np.zeros((B, S, NSAMP), np.float32)
    for i in range(NCORES):
        r = res.results[i]
        new_xyz[i * SPC:(i + 1) * SPC] = \
            r["newxyz"][::G].reshape(SPC, S, 3)
        vout[i * SPC:(i + 1) * SPC] = r["vout"].reshape(SPC, S, NSAMP)
    # slot 0 center comes from host (point 0 of each sample)
    new_xyz[:, 0, :] = pc[:, :, 0]
    idx = _decode_neighbors(vout)
    if return_raw:
        return new_xyz, idx, res
    return new_xyz, idx


def kernel(**inputs):
    pc = np.ascontiguousarray(inputs["pc_electrode"], dtype=np.float32)
    new_xyz, idx = run_device(pc)
    return _host_head(pc, new_xyz, idx, inputs)


# revision 31
# speedup vs baseline: 1.0698x; 1.0129x over previous
"""Trainium2 Bass kernel for nn_DeepSSM_Net (PointNet++-style SSM head).

Pure data parallel: B=128 samples, 16 per core across 8 NeuronCores.
Per core: xyz planes SBUF-resident as [128 partitions = 16 samples x 8
groups, 8192 points]. Fused FPS (10 centers) + ball query:

Per step, per 1024-col chunk:
  - ScalarE: 3 Square activations (x-cx)^2 etc (bias = -c per partition)
  - GpSimd:  sqx+sqy (scalar_tensor_tensor), half the code passes
  - VectorE: d = (sqx+sqy)+sqz, fused min-update + chunk-max
    (tensor_tensor_reduce), per-chunk argmax index (max_index), top-8
    in-radius codes (InstMax on f16 codes).
Argmax resolution across the 8 partition-groups of a sample is done
on-chip with DVE 32x32 stream transposes (no DRAM bounce); the winner's
coords are gathered per-partition with one indirect DMA per coordinate,
which also broadcasts the new center to all 8 group partitions.

Ball-query codes: per chunk top-8 of (d <= R2) * (1024-col) in f16,
merged at the end into per-sample first-8-by-index codes (f32 integer
code space), decoded to indices on host. Tiny MLP/BN/FC head runs on
host (~0.003% of FLOPs, couples samples across cores through batch BN).
"""

import numpy as np

# problem constants (hardcoded per the task contract)
B, N, S, NSAMP = 128, 65536, 10, 8
NCORES, SPC = 8, 16          # cores, samples per core
G, FPP = 8, 8192             # partition-groups per sample, points/partition
CH, NCH = 1024, 8            # chunk columns, chunks per row
R2 = float(np.float32(0.04))
CSTEP = 1025                 # chunk-code stride (code in [1,1024], 0 empty)
GSTEP = 8201                 # group-code stride (chunk code in [1,8200])
MAXF = float(B * FPP)        # 1048576: > any flat index
WBIG = -3.0e38

_CACHE = {}


def _build_program():
    import os
    import concourse.bass as bass
    import concourse.tile as tile
    from concourse import bacc, mybir
    from concourse._compat import with_exitstack

    stage = int(os.environ.get("KERNEL_STAGE", "4"))
    chunk_ops = int(os.environ.get("KERNEL_CHUNK_OPS", "3"))

    F32 = mybir.dt.float32
    F16 = mybir.dt.float16
    U16 = mybir.dt.uint16
    U32 = mybir.dt.uint32
    Alu = mybir.AluOpType
    Act = mybir.ActivationFunctionType
    Ax = mybir.AxisListType

    nc = bacc.Bacc("TRN2", target_bir_lowering=False, debug=False,
                   num_devices=1)

    xd = nc.dram_tensor("xd", [128, FPP], F32, kind="ExternalInput")
    yd = nc.dram_tensor("yd", [128, FPP], F32, kind="ExternalInput")
    zd = nc.dram_tensor("zd", [128, FPP], F32, kind="ExternalInput")
    idxvc_d = nc.dram_tensor("idxvc", [128, CH], F16, kind="ExternalInput")
    cs0_d = nc.dram_tensor("cs0", [128, 3], F32, kind="ExternalInput")
    iota8_d = nc.dram_tensor("iota8", [128, G], F32, kind="ExternalInput")
    pbase_d = nc.dram_tensor("pbase", [128, 1], F32, kind="ExternalInput")
    addc_d = nc.dram_tensor("addc", [128, NCH * NSAMP], F32,
                            kind="ExternalInput")
    gofs_d = nc.dram_tensor("gofs", [128, 1], F32, kind="ExternalInput")
    newxyz_d = nc.dram_tensor("newxyz", [128, 3 * S], F32,
                              kind="ExternalOutput")
    vout_d = nc.dram_tensor("vout", [SPC, NSAMP * S], F32,
                            kind="ExternalOutput")

    @with_exitstack
    def prog(ctx, tc):
        big = ctx.enter_context(tc.tile_pool(name="big", bufs=1))
        scr = ctx.enter_context(tc.tile_pool(name="scr", bufs=2))
        cod = ctx.enter_context(tc.tile_pool(name="cod", bufs=2))
        sml = ctx.enter_context(tc.tile_pool(name="sml", bufs=2))
        cst = ctx.enter_context(tc.tile_pool(name="cst", bufs=1))
        csp = ctx.enter_context(tc.tile_pool(name="csp", bufs=2))
        drm = ctx.enter_context(tc.tile_pool(name="drm", bufs=1,
                                             space="DRAM"))

        # pad between big tiles: breaks 32KB address aliasing between the
        # 2-read+1-write streams of the min op (measured 3x slowdown when
        # DD/DST are exactly 32KB apart)
        X = big.tile([128, FPP], F32, tag="X",
                     padded_shape=[128, FPP + 136])
        Y = big.tile([128, FPP], F32, tag="Y",
                     padded_shape=[128, FPP + 104])
        Z = big.tile([128, FPP], F32, tag="Z",
                     padded_shape=[128, FPP + 136])
        DST = big.tile([128, FPP], F32, tag="DST",
                       padded_shape=[128, FPP + 104])
        DD = big.tile([128, FPP], F32, tag="DD",
                      padded_shape=[128, FPP + 136])
        nc.sync.dma_start(X[:], xd.ap())
        nc.sync.dma_start(Y[:], yd.ap())
        nc.sync.dma_start(Z[:], zd.ap())
        nc.vector.memset(DST[:], 1.0e10)

        idxvc = cst.tile([128, CH], F16, tag="idxvc")
        nc.sync.dma_start(idxvc[:], idxvc_d.ap())
        iota8 = cst.tile([128, G], F32, tag="iota8")
        nc.sync.dma_start(iota8[:], iota8_d.ap())
        pbase = cst.tile([128, 1], F32, tag="pbase")
        nc.sync.dma_start(pbase[:], pbase_d.ap())
        addc = cst.tile([128, NCH * NSAMP], F32, tag="addc")
        nc.sync.dma_start(addc[:], addc_d.ap())
        gofs = cst.tile([128, 1], F32, tag="gofs")
        nc.sync.dma_start(gofs[:], gofs_d.ap())

        ones8 = cst.tile([128, G], F32, tag="ones8")
        nc.vector.memset(ones8[:], 1.0)
        r2c = cst.tile([128, 1], F32, tag="r2c")
        nc.vector.memset(r2c[:], R2)
        vt8 = cst.tile([128, S * NCH * NSAMP], F16, tag="vt8")
        p8all = cst.tile([128, S * NSAMP], F32, tag="p8all")
        nxyz = cst.tile([128, 3 * S], F32, tag="nxyz")
        nc.vector.memset(nxyz[:, 0:3], 0.0)
        pack = cst.tile([128, 64], F32, tag="pack")
        nc.vector.memset(pack[:], 0.0)
        packB = cst.tile([128, 32], F32, tag="packB")

        cs0 = csp.tile([128, 3], F32, tag="csneg")
        nc.sync.dma_start(cs0[:], cs0_d.ap())
        csneg = cs0

        xyz_flat = [
            bass.AP(t.ap().tensor, 0, [[1, 128 * FPP], [1, 1]])
            for t in (xd, yd, zd)
        ]

        def emit_bq_chunk(k, j):
            # ball-query codes + top8 for (step k, chunk j); reads DD.
            # sign(R2 - d) on ScalarE, f16 mul on V: measured faster than
            # the fused STT form (801us vs 842us wall)
            sl = slice(j * CH, (j + 1) * CH)
            V16 = cod.tile([128, CH], F16, tag="V16",
                           padded_shape=[128, CH + 16])
            SG = cod.tile([128, CH], F16, tag="SG",
                          padded_shape=[128, CH + 48])
            nc.scalar.activation(SG[:], DD[:, sl], Act.Sign,
                                 bias=r2c[:, 0:1], scale=-1.0)
            nc.vector.tensor_tensor(V16[:], SG[:], idxvc[:], Alu.mult)
            nc.vector.max(vt8[:, (k * NCH + j) * 8:(k * NCH + j) * 8 + 8],
                          V16[:])

        for k in range(S):
            cm8 = sml.tile([128, NCH * 8], F32, tag="cm8")
            cidx = sml.tile([128, NCH * 8], U16, tag="cidx")
            for j in range(NCH):
                sl = slice(j * CH, (j + 1) * CH)
                A = scr.tile([128, CH], F32, tag="A",
                             padded_shape=[128, CH + 8])
                Bt = scr.tile([128, CH], F32, tag="B",
                              padded_shape=[128, CH + 16])
                Ct = scr.tile([128, CH], F32, tag="C",
                              padded_shape=[128, CH + 8])
                nc.scalar.activation(A[:], X[:, sl], Act.Square,
                                     bias=csneg[:, 0:1])
                nc.scalar.activation(Bt[:], Y[:, sl], Act.Square,
                                     bias=csneg[:, 1:2])
                nc.scalar.activation(Ct[:], Z[:, sl], Act.Square,
                                     bias=csneg[:, 2:3])
                # software pipeline: step k-1's ball query for this chunk
                # runs here, before this step's Pool-d overwrites DD[:, sl]
                # (Tile serializes via the WAR hazard on DD automatically);
                # keeps V busy through the argmax-tail window.
                if k > 0 and stage >= 3:
                    emit_bq_chunk(k - 1, j)
                nc.gpsimd.tensor_tensor(A[:], A[:], Bt[:], Alu.add)
                nc.gpsimd.tensor_tensor(DD[:, sl], A[:], Ct[:], Alu.add)
                if chunk_ops < 2:
                    continue
                # dist = min(d, dist)
                nc.vector.tensor_tensor(DST[:, sl], DD[:, sl], DST[:, sl],
                                        Alu.min)
                if chunk_ops < 3:
                    continue
                # chunk max + its in-chunk index
                nc.vector.max(cm8[:, j * 8:(j + 1) * 8], DST[:, sl])
                nc.vector.max_index(cidx[:, j * 8:(j + 1) * 8],
                                    cm8[:, j * 8:(j + 1) * 8], DST[:, sl])
            if k == S - 1:
                if stage >= 3:
                    for j in range(NCH):
                        emit_bq_chunk(k, j)
                break
            if stage < 2:
                continue
            # ---- argmax tail: resolve winner across chunks+groups ----
            cmax = cm8.rearrange("p (j e) -> p j e", e=8)[:, :, 0]
            rmax = sml.tile([128, 1], F32, tag="rmax")
            nc.vector.tensor_reduce(rmax[:], cmax, Ax.X, Alu.max)
            rm8b = sml.tile([128, 8], F32, tag="rm8b")
            nc.gpsimd.tensor_scalar(rm8b[:], ones8[:], rmax[:, 0:1], None,
                                    op0=Alu.mult)
            cfind = sml.tile([128, 8], U16, tag="cfind")
            nc.vector.max_index(cfind[:], rm8b[:], cmax)
            cf = sml.tile([128, 1], F32, tag="cf")
            nc.vector.tensor_copy(cf[:], cfind[:, 0:1])
            oh = sml.tile([128, 8], F32, tag="oh")
            nc.vector.tensor_scalar(oh[:], iota8[:], cf[:, 0:1], None,
                                    op0=Alu.is_equal)
            cidxf = sml.tile([128, 8], F32, tag="cidxf")
            nc.vector.tensor_copy(
                cidxf[:], cidx.rearrange("p (j e) -> p j e", e=8)[:, :, 0])
            ohp = sml.tile([128, 8], F32, tag="ohp")
            idxin = sml.tile([128, 1], F32, tag="idxin")
            nc.vector.tensor_tensor(ohp[:], oh[:], cidxf[:], Alu.mult)
            nc.vector.tensor_reduce(idxin[:], ohp[:], Ax.X, Alu.add)
            flat = sml.tile([128, 1], F32, tag="flat")
            nc.vector.scalar_tensor_tensor(flat[:], cf[:], float(CH),
                                           idxin[:], Alu.mult, Alu.add)
            nc.vector.tensor_tensor(flat[:], flat[:], pbase[:], Alu.add)
            nc.vector.tensor_copy(pack[:, 0:1], rmax[:])
            nc.vector.tensor_copy(pack[:, 32:33], flat[:])
            T = sml.tile([128, 64], F32, tag="T")
            nc.vector.transpose(T[:], pack[:])
            Tv = T.rearrange("p (q r) -> p q r", r=8)
            smax4 = sml.tile([128, 4], F32, tag="smax4")
            nc.vector.tensor_reduce(smax4[:], Tv[:, 0:4, :], Ax.X, Alu.max)
            smaxb = sml.tile([128, 32], F32, tag="smaxb")
            sbv = smaxb.rearrange("p (q r) -> p q r", r=8)
            for r in range(8):
                nc.vector.tensor_copy(sbv[:, :, r], smax4[:])
            mask = sml.tile([128, 32], F32, tag="mask")
            nc.vector.tensor_tensor(mask[:], T[:, 0:32], smaxb[:], Alu.is_ge)
            sub = sml.tile([128, 32], F32, tag="sub")
            nc.vector.tensor_scalar(sub[:], T[:, 32:64], -1.0, MAXF,
                                    op0=Alu.mult, op1=Alu.add)
            enc = sml.tile([128, 32], F32, tag="enc")
            nc.vector.tensor_mul(enc[:], mask[:], sub[:])
            e4 = sml.tile([128, 4], F32, tag="e4")
            nc.vector.tensor_reduce(
                e4[:], enc.rearrange("p (q r) -> p q r", r=8)[:], Ax.X,
                Alu.max)
            wf4 = sml.tile([128, 4], F32, tag="wf4")
            nc.vector.tensor_scalar(wf4[:], e4[:], -1.0, MAXF,
                                    op0=Alu.mult, op1=Alu.add)
            pbv = packB.rearrange("p (q r) -> p q r", r=8)
            for r in range(8):
                nc.vector.tensor_copy(pbv[:, :, r], wf4[:])
            TB = sml.tile([128, 32], F32, tag="TB")
            nc.vector.transpose(TB[:], packB[:])
            flatu = sml.tile([128, 1], U32, tag="flatu")
            nc.vector.tensor_copy(flatu[:], TB[:, 0:1])
            cs = csp.tile([128, 3], F32, tag="cs")
            for c, fl in enumerate(xyz_flat):
                nc.gpsimd.indirect_dma_start(
                    cs[:, c:c + 1], None, fl,
                    bass.IndirectOffsetOnAxis(ap=flatu[:], axis=0))
            col = 3 * (k + 1)
            nc.vector.tensor_copy(nxyz[:, col:col + 3], cs[:])
            csneg = csp.tile([128, 3], F32, tag="csneg")
            nc.vector.tensor_scalar(csneg[:], cs[:], -1.0, None, op0=Alu.mult)

        # ---- ball-query merge ----
        if stage < 4:
            vout0 = cst.tile([SPC, NSAMP * S], F32, tag="vout")
            nc.vector.memset(vout0[:], 0.0)
            nc.sync.dma_start(newxyz_d.ap(), nxyz[:])
            nc.sync.dma_start(vout_d.ap(), vout0[:])
            return
        for k in range(S):
            w64 = sml.tile([128, 64], F32, tag="w64")
            nc.vector.tensor_copy(w64[:], vt8[:, k * 64:(k + 1) * 64])
            t64 = sml.tile([128, 64], F32, tag="t64")
            nc.vector.tensor_tensor(t64[:], w64[:], addc[:], Alu.add)
            g64 = sml.tile([128, 64], F32, tag="g64")
            nc.vector.scalar_tensor_tensor(g64[:], w64[:], 0.0, t64[:],
                                           Alu.is_gt, Alu.mult)
            p8k = sml.tile([128, 8], F32, tag="p8k")
            nc.vector.max(p8k[:], g64[:])
            tp8 = sml.tile([128, 8], F32, tag="tp8")
            nc.vector.tensor_scalar(tp8[:], p8k[:], gofs[:, 0:1], None,
                                    op0=Alu.add)
            nc.vector.scalar_tensor_tensor(p8all[:, k * 8:k * 8 + 8], p8k[:],
                                           0.0, tp8[:], Alu.is_gt, Alu.mult)
        dp8 = drm.tile([128, S * NSAMP], F32, tag="dp8")
        nc.sync.dma_start(dp8[:], p8all[:])
        sc = cst.tile([SPC, G * S * NSAMP], F32, tag="sc")
        nc.sync.dma_start(sc[:], dp8.rearrange("(s g) c -> s (g c)", g=G))
        scv = sc.rearrange("s (g c) -> s g c", c=S * NSAMP)
        vout = cst.tile([SPC, NSAMP * S], F32, tag="vout")
        for k in range(S):
            nc.vector.max(vout[:, k * 8:k * 8 + 8],
                          scv[:, :, k * 8:k * 8 + 8])
        nc.sync.dma_start(newxyz_d.ap(), nxyz[:])
        nc.sync.dma_start(vout_d.ap(), vout[:])

    with tile.TileContext(nc) as tc:
        prog(tc)
    nc.compile()
    return nc


def _get_nc():
    if "nc" not in _CACHE:
        _CACHE["nc"] = _build_program()
    return _CACHE["nc"]


def _make_consts():
    idxvc = np.broadcast_to(
        (CH - np.arange(CH, dtype=np.float16))[None, :].astype(np.float16),
        (128, CH)).copy()
    iota8 = np.broadcast_to(
        np.arange(G, dtype=np.float32)[None, :], (128, G)).copy()
    pbase = (np.arange(128, dtype=np.float32) * FPP)[:, None].copy()
    cols = np.arange(NCH * NSAMP)
    addc = np.broadcast_to(
        ((NCH - 1 - cols // NSAMP) * CSTEP).astype(np.float32)[None, :],
        (128, NCH * NSAMP)).copy()
    gofs = ((G - 1 - np.arange(128) % G) * GSTEP).astype(
        np.float32)[:, None].copy()
    return idxvc, iota8, pbase, addc, gofs


def _make_in_maps(pc):
    idxvc, iota8, pbase, addc, gofs = _make_consts()
    in_maps = []
    for i in range(NCORES):
        shard = pc[i * SPC:(i + 1) * SPC]          # [16, 3, 65536]
        planes = [np.ascontiguousarray(
            shard[:, c, :].reshape(128, FPP)) for c in range(3)]
        p0 = shard[:, :, 0]                        # [16, 3]
        cs0 = np.repeat(-p0, G, axis=0).astype(np.float32)   # [128, 3]
        in_maps.append({
            "xd": planes[0], "yd": planes[1], "zd": planes[2],
            "idxvc": idxvc, "cs0": cs0, "iota8": iota8,
            "pbase": pbase, "addc": addc, "gofs": gofs,
        })
    return in_maps


def _decode_neighbors(vout):
    """vout: [B, S, 8] merged sample-codes -> idx [B, S, 8] int32."""
    u = np.rint(vout).astype(np.int64)
    g = (G - 1) - (np.maximum(u, 1) - 1) // GSTEP
    rem = u - (G - 1 - g) * GSTEP
    j = (NCH - 1) - (np.maximum(rem, 1) - 1) // CSTEP
    code = rem - (NCH - 1 - j) * CSTEP
    col = CH - code
    n = g * FPP + j * CH + col
    empty = u == 0
    n = np.where(empty, n[:, :, 0:1], n)
    return n.astype(np.int32)


def _host_head(pc, new_xyz, idx, p):
    """grouping + shared MLP + BN + FC head (numpy, float64 accum)."""
    xyz = pc.transpose(0, 2, 1).astype(np.float64)       # [B, N, 3]
    bi = np.arange(B)[:, None, None]
    grouped = xyz[bi, idx]                               # [B, S, 8, 3]
    grouped = grouped - new_xyz[:, :, None, :].astype(np.float64)
    x = grouped.transpose(0, 3, 2, 1)                    # [B, 3, 8, S]

    def bn(v, g, be):
        m = v.mean(axis=(0, 2, 3), keepdims=True)
        var = v.var(axis=(0, 2, 3), keepdims=True)
        return (v - m) / np.sqrt(var + 1e-5) * g[None, :, None, None] \
            + be[None, :, None, None]

    for w, b, g, be in (("w1", "b1", "g1", "be1"), ("w2", "b2", "g2", "be2"),
                        ("w3", "b3", "g3", "be3")):
        w, b, g, be = (p[w].astype(np.float64), p[b].astype(np.float64),
                       p[g].astype(np.float64), p[be].astype(np.float64))
        x = np.einsum("oc,bcns->bons", w, x) + b[None, :, None, None]
        x = np.maximum(bn(x, g, be), 0.0)
    x = x.max(axis=2)                                    # [B, 16, S]
    feat = x.reshape(B, -1)
    h = feat @ p["fc1_w"].astype(np.float64).T + p["fc1_b"].astype(np.float64)
    m = h.mean(0, keepdims=True)
    v = h.var(0, keepdims=True)
    h = (h - m) / np.sqrt(v + 1e-5) * p["bn1_g"].astype(np.float64) \
        + p["bn1_b"].astype(np.float64)
    h = np.maximum(h, 0.0)
    out = h @ p["fc2_w"].astype(np.float64).T + p["fc2_b"].astype(np.float64)
    return out.astype(np.float32)


def run_device(pc, trace=False, return_raw=False):
    """Returns (new_xyz [B,S,3] f32, idx [B,S,8] i32) from the 8-core run."""
    from concourse import bass_utils
    nc = _get_nc()
    in_maps = _make_in_maps(pc)
    res = bass_utils.run_bass_kernel_spmd(nc, in_maps,
                                          core_ids=list(range(NCORES)),
                                          trace=trace)
    new_xyz = np.zeros((B, S, 3), np.float32)
    vout = np.zeros((B, S, NSAMP), np.float32)
    for i in range(NCORES):
        r = res.results[i]
        new_xyz[i * SPC:(i + 1) * SPC] = \
            r["newxyz"][::G].reshape(SPC, S, 3)
        vout[i * SPC:(i + 1) * SPC] = r["vout"].reshape(SPC, S, NSAMP)
    # slot 0 center comes from host (point 0 of each sample)
    new_xyz[:, 0, :] = pc[:, :, 0]
    idx = _decode_neighbors(vout)
    if return_raw:
        return new_xyz, idx, res
    return new_xyz, idx


def kernel(**inputs):
    pc = np.ascontiguousarray(inputs["pc_electrode"], dtype=np.float32)
    new_xyz, idx = run_device(pc)
    return _host_head(pc, new_xyz, idx, inputs)
